# revision 3
# baseline (speedup 1.0000x reference)
"""Gated axial attention (width axis) Trainium2 Bass kernel, v5.

v5 = v4 + fp8(e4m3) DoubleRow V-projection: x and g_v1*v_w are
quantized to fp8 (weights prescaled by 32, descaled in the fused DVE
scalar_tensor_tensor that also adds pos_v), and each V chain becomes 2
K=256 DoubleRow matmuls instead of 4 K=128 bf16 matmuls. Costs ~1.4e-2
relative error (gate 2e-2), saves ~28us of PE time.

v4 notes:

Data-parallel over the fused B*H row axis (512 rows -> 64 per core).

v4 = v3 algorithms (2-matmul chunk-layout scores with precomputed
K' = k + gq*pq + kb; parity PSUM split; ones-column softmax denominator)
with a fully software-pipelined global schedule. Work is organized in
half-blocks of 8 rows; at each global row we issue 3 projection chains
(Q/K/V) of the NEXT half-block plus this row's scores, the previous
row's AV, the transpose two rows back, and an o-projection chain per
4-row group. The PE therefore always has an independent matmul chain
in flight and never stalls on ACT's exp latency. This matters twice:
once for the stall itself, and once because the PE DVFS p-state
(0.65/1.2/2.4 GHz) resets on every pipeline gap and takes ~3us of
continuous execution to return to full clock.

Engine assignment: ACT = proj copies + exp + oproj copies; DVE = K'
adds + vmix adds + softmax normalize + transpose copybacks.

Scale folding (host): 1/sqrt(hd) into q_w/q_b; g_q into pos_q (+k_b
into the replicated pq table); g_k*scale into pos_k; g_v1 into
v_w/v_b; g_v2 into pos_v; o_b added on host.
"""

import sys
import types

sys.path.insert(0, "/opt/trn_rl_repo")


def _install_ntff_shim():
    """Make bass_utils trace=True work under axon (BASS_TRACE=1)."""
    try:
        import antenv
    except ImportError:
        return
    if "antenv.axon_hooks" in sys.modules:
        return
    mod = types.ModuleType("antenv.axon_hooks")
    _hook = [None]

    def set_axon_ntff_profile_hook(h):
        _hook[0] = h

    def get_axon_ntff_profile_hook():
        if _hook[0] is None:
            try:
                if "/root/.axon_site" not in sys.path:
                    sys.path.insert(0, "/root/.axon_site")
                from trn_agent_boot.trn_boot import _ntff_profile_via_ctypes

                _hook[0] = _ntff_profile_via_ctypes("/opt/axon/libaxon_pjrt.so")
            except Exception:
                _hook[0] = None
        return _hook[0]

    mod.set_axon_ntff_profile_hook = set_axon_ntff_profile_hook
    mod.get_axon_ntff_profile_hook = get_axon_ntff_profile_hook
    sys.modules["antenv.axon_hooks"] = mod
    antenv.axon_hooks = mod


_install_ntff_shim()

import ml_dtypes  # noqa: E402
import numpy as np  # noqa: E402

import concourse.bass as bass  # noqa: E402
import concourse.tile as tile  # noqa: E402
from concourse import bacc, mybir  # noqa: E402
from concourse.bass_utils import run_bass_kernel_spmd  # noqa: E402

BF16 = ml_dtypes.bfloat16

B, C, H, W = 4, 512, 128, 128
NH, HD = 8, 64
NCORES = 8
ROWS = B * H
RPC = ROWS // NCORES  # 64 rows per core
HB = 8  # rows per half-block
NHB = RPC // HB  # 8
P = 128
NCH = C // P
THB = HB * W  # tokens per half-block (1024)
NTT = THB // 512  # 2

_CACHED_NC = None
LAST_RESULTS = None


def _build_nc():
    nc = bacc.Bacc("TRN2", target_bir_lowering=False, debug=False,
                   num_devices=NCORES)
    dt = mybir.dt
    ID = mybir.ActivationFunctionType.Identity
    EXP = mybir.ActivationFunctionType.Exp

    xt = nc.dram_tensor("xt", [NCH, P, RPC, W], dt.bfloat16,
                        kind="ExternalInput")
    xt8 = nc.dram_tensor("xt8", [NCH, P, RPC, W], dt.float8e4,
                         kind="ExternalInput")
    vw8 = nc.dram_tensor("vw8", [C, C], dt.float8e4, kind="ExternalInput")
    q_wt = nc.dram_tensor("q_wt", [C, C], dt.bfloat16, kind="ExternalInput")
    k_wt = nc.dram_tensor("k_wt", [C, C], dt.bfloat16, kind="ExternalInput")
    v_wt = nc.dram_tensor("v_wt", [C, C], dt.bfloat16, kind="ExternalInput")
    o_wt = nc.dram_tensor("o_wt", [C, C], dt.bfloat16, kind="ExternalInput")
    qb = nc.dram_tensor("qb", [C], dt.float32, kind="ExternalInput")
    kb = nc.dram_tensor("kb", [C], dt.float32, kind="ExternalInput")
    pqr = nc.dram_tensor("pqr", [NCH, P, 512], dt.bfloat16,
                         kind="ExternalInput")
    pkts = nc.dram_tensor("pkts", [NCH, P, W], dt.bfloat16,
                          kind="ExternalInput")
    pvs = nc.dram_tensor("pvs", [W, C], dt.bfloat16, kind="ExternalInput")
    ident = nc.dram_tensor("ident", [P, P], dt.bfloat16, kind="ExternalInput")
    out_t = nc.dram_tensor("out_t", [NCH, P, RPC, W], dt.float32,
                           kind="ExternalOutput")

    with tile.TileContext(nc) as tc:
        with (
            tc.tile_pool(name="const", bufs=1) as const,
            tc.tile_pool(name="xtp", bufs=2) as xtp,
            tc.tile_pool(name="xtp8", bufs=2) as xtp8,
            tc.tile_pool(name="qkp", bufs=2) as qkp,
            tc.tile_pool(name="vmixp", bufs=2) as vmixp,
            tc.tile_pool(name="expp", bufs=4) as expp,
            tc.tile_pool(name="aop", bufs=4) as aop,
            tc.tile_pool(name="aotp", bufs=2) as aotp,
            tc.tile_pool(name="small", bufs=4) as small,
            tc.tile_pool(name="fop", bufs=2) as fop,
            tc.tile_pool(name="ps_proj", bufs=3, space="PSUM") as ps_proj,
            tc.tile_pool(name="ps_sc", bufs=2, space="PSUM") as ps_sc,
            tc.tile_pool(name="ps_av", bufs=2, space="PSUM") as ps_av,
            tc.tile_pool(name="ps_tr", bufs=1, space="PSUM") as ps_tr,
        ):
            # ---- constants into SBUF ----
            def load_w(name, dram, eng):
                t = const.tile([P, NCH, C], dt.bfloat16, name=name)
                eng.dma_start(out=t,
                              in_=dram.ap().rearrange("(k p) c -> p k c",
                                                      p=P))
                return t

            qw_sb = load_w("qw_sb", q_wt, nc.gpsimd)
            kw_sb = load_w("kw_sb", k_wt, nc.scalar)
            vw8_sb = const.tile([P, NCH, C], dt.float8e4)
            nc.gpsimd.dma_start(out=vw8_sb,
                                in_=vw8.ap().rearrange("(k p) c -> p k c",
                                                       p=P))
            vw_sb = None  # unused with fp8 V projection

            pqr_sb = const.tile([P, NCH, 512], dt.bfloat16)
            nc.scalar.dma_start(out=pqr_sb,
                                in_=pqr.ap().rearrange("m p t -> p m t"))
            pk_sb = const.tile([P, NCH, W], dt.bfloat16)
            nc.gpsimd.dma_start(out=pk_sb,
                                in_=pkts.ap().rearrange("k p w -> p k w"))
            pv_sb = const.tile([P, C], dt.bfloat16)
            nc.gpsimd.dma_start(out=pv_sb, in_=pvs.ap())
            id_sb = const.tile([P, P], dt.bfloat16)
            nc.gpsimd.dma_start(out=id_sb, in_=ident.ap())
            qb_sb = const.tile([P, NCH], dt.float32)
            nc.gpsimd.dma_start(out=qb_sb,
                                in_=qb.ap().rearrange("(m p) -> p m", p=P))
            kb_sb = const.tile([P, NCH], dt.float32)
            nc.gpsimd.dma_start(out=kb_sb,
                                in_=kb.ap().rearrange("(m p) -> p m", p=P))
            ow_sb = load_w("ow_sb", o_wt, nc.gpsimd)

            xt_r = xt.ap()
            xt8_r = xt8.ap()
            out_r = out_t.ap()

            # per-half-block tile handles
            xts = [None] * NHB
            xt8s = [None] * NHB
            qts = [None] * NHB
            kts = [None] * NHB
            kpqs = [None] * NHB
            vmixs = [None] * NHB
            exs = [None] * RPC
            aos = [None] * RPC
            aots = [None] * (RPC // 4)

            def load_xt(hb):
                xts[hb] = xtp.tile([P, NCH, THB], dt.bfloat16, tag="xt",
                                   name="xt_sb")
                xt8s[hb] = xtp8.tile([P, NCH, THB], dt.float8e4, tag="xt8",
                                     name="xt8_sb")
                r0 = hb * HB
                nc.sync.dma_start(out=xts[hb],
                                  in_=xt_r[:, :, r0:r0 + HB, :]
                                  .rearrange("k p r w -> p k (r w)"))
                nc.sync.dma_start(out=xt8s[hb],
                                  in_=xt8_r[:, :, r0:r0 + HB, :]
                                  .rearrange("k p r w -> p k (r w)"))

            def alloc_qk(hb):
                qts[hb] = [qkp.tile([P, NCH, 512], dt.bfloat16,
                                    tag=f"qt{n}", name="qt_sb")
                           for n in range(NTT)]
                kts[hb] = [qkp.tile([P, NCH, 512], dt.bfloat16,
                                    tag=f"kt{n}", name="kt_sb")
                           for n in range(NTT)]
                kpqs[hb] = [qkp.tile([P, NCH, 512], dt.bfloat16,
                                     tag=f"kpq{n}", name="kpq_sb")
                            for n in range(NTT)]

            def alloc_vmix(hb):
                vmixs[hb] = vmixp.tile([P, HB, NH * 65], dt.bfloat16,
                                       tag="vmix", name="vmix")
                nc.vector.memset(
                    vmixs[hb].rearrange("p r (h e) -> p r h e", e=65)
                    [:, :, :, 64:65], 1.0)

            def qk_chains(hb, q_on_dve=False):
                """16 thunks: pairs (Q(m,n), K(m,n)), n-major so the
                n=0 tiles (rows 0-3) finish draining early."""
                xt_sb = xts[hb]
                thunks = []
                for i in range(8):
                    n, m = i // NCH, i % NCH
                    nt = slice(n * 512, (n + 1) * 512)

                    def qchain(m=m, n=n, nt=nt):
                        ps = ps_proj.tile([P, 512], dt.float32, tag="pp",
                                          name="ps")
                        for k in range(NCH):
                            nc.tensor.matmul(
                                ps,
                                lhsT=qw_sb[:, k, m * P:(m + 1) * P],
                                rhs=xt_sb[:, k, nt],
                                start=(k == 0), stop=(k == NCH - 1))
                        if q_on_dve:
                            nc.vector.tensor_scalar_add(
                                qts[hb][n][:, m, :], in0=ps,
                                scalar1=qb_sb[:, m:m + 1])
                        else:
                            nc.scalar.activation(qts[hb][n][:, m, :], ps, ID,
                                                 bias=qb_sb[:, m:m + 1])

                    def kchain(m=m, n=n, nt=nt):
                        ps = ps_proj.tile([P, 512], dt.float32, tag="pp",
                                          name="ps")
                        for k in range(NCH):
                            nc.tensor.matmul(
                                ps,
                                lhsT=kw_sb[:, k, m * P:(m + 1) * P],
                                rhs=xt_sb[:, k, nt],
                                start=(k == 0), stop=(k == NCH - 1))
                        nc.scalar.activation(kts[hb][n][:, m, :], ps, ID,
                                             bias=kb_sb[:, m:m + 1])
                        nc.vector.tensor_add(out=kpqs[hb][n][:, m, :],
                                             in0=ps, in1=pqr_sb[:, m, :])

                    thunks += [qchain, kchain]
                return thunks

            def vchain(hb, r):
                xt8_sb = xt8s[hb]
                ps = ps_proj.tile([P, 512], dt.float32, tag="pp",
                                  name="ps")
                for s in range(2):
                    nc.tensor.matmul(
                        ps,
                        lhsT=xt8_sb[:, 2 * s:2 * s + 2, r * P:(r + 1) * P],
                        rhs=vw8_sb[:, 2 * s:2 * s + 2, :],
                        perf_mode=mybir.MatmulPerfMode.DoubleRow,
                        start=(s == 0), stop=(s == 1))
                nc.vector.scalar_tensor_tensor(
                    out=vmixs[hb][:, r, :].rearrange(
                        "p (h e) -> p h e", e=65)[:, :, 0:64],
                    in0=ps.rearrange("p (h e) -> p h e", e=64),
                    scalar=1.0 / 32.0,
                    in1=pv_sb.rearrange("p (h e) -> p h e", e=64),
                    op0=mybir.AluOpType.mult,
                    op1=mybir.AluOpType.add)

            def scores(gr):
                hb, r = divmod(gr, HB)
                n, rr = divmod(r, 4)
                rt = slice(rr * P, (rr + 1) * P)
                ex = expp.tile([P, 2, 512], dt.bfloat16, tag="ex", name="ex")
                for par in range(2):
                    lo = par * HD
                    hi = lo + HD
                    pss = ps_sc.tile([P, 512], dt.float32, tag="sc",
                                     name="pss")
                    for idx in range(4):
                        dst = pss[:, idx * P:(idx + 1) * P]
                        nc.tensor.matmul(
                            dst,
                            lhsT=kpqs[hb][n][lo:hi, idx, rt],
                            rhs=qts[hb][n][lo:hi, idx, rt],
                            start=True, stop=False)
                        nc.tensor.matmul(
                            dst,
                            lhsT=kts[hb][n][lo:hi, idx, rt],
                            rhs=pk_sb[lo:hi, idx, :],
                            start=False, stop=True)
                    nc.scalar.activation(ex[:, par, :], pss, EXP)
                exs[gr] = ex

            def av(gr):
                hb, r = divmod(gr, HB)
                ex = exs[gr]
                ao = aop.tile([P, C], dt.bfloat16, tag="ao", name="ao")
                for par in range(2):
                    psa = ps_av.tile([P, 4 * 65], dt.float32, tag="av",
                                     name="psa")
                    for idx in range(4):
                        h = 2 * idx + par
                        nc.tensor.matmul(
                            psa[:, idx * 65:(idx + 1) * 65],
                            lhsT=ex[:, par, idx * P:(idx + 1) * P],
                            rhs=vmixs[hb][:, r, h * 65:(h + 1) * 65],
                            start=True, stop=True)
                    rv = small.tile([P, 4, 1], dt.float32, tag="rv",
                                    name="rv")
                    nc.vector.reciprocal(
                        rv,
                        psa.rearrange("p (h e) -> p h e", e=65)[:, :, 64:65])
                    for idx in range(4):
                        h = 2 * idx + par
                        nc.vector.tensor_scalar_mul(
                            ao[:, h * HD:(h + 1) * HD],
                            in0=psa[:, idx * 65:idx * 65 + 64],
                            scalar1=rv[:, idx, :])
                aos[gr] = ao

            def transpose(gr):
                g, rr = divmod(gr, 4)
                if rr == 0:
                    aots[g] = aotp.tile([P, NCH, 512], dt.bfloat16,
                                        tag="aot", name="aot")
                ao = aos[gr]
                pst = ps_tr.tile([P, NCH, P], dt.bfloat16, tag="tr",
                                 name="pst")
                for ch in range(NCH):
                    nc.tensor.transpose(
                        pst[:, ch, :], ao[:, ch * P:(ch + 1) * P], id_sb)
                nc.vector.tensor_copy(
                    aots[g][:, :, rr * P:(rr + 1) * P], pst)

            def oproj_unit(g, m):
                ps = ps_proj.tile([P, 512], dt.float32, tag="pp",
                                  name="ps")
                for k in range(NCH):
                    nc.tensor.matmul(
                        ps,
                        lhsT=ow_sb[:, k, m * P:(m + 1) * P],
                        rhs=aots[g][:, k, :],
                        start=(k == 0), stop=(k == NCH - 1))
                fo = fop.tile([P, 512], dt.float32, tag="fo", name="fo")
                nc.scalar.copy(fo, ps)
                nc.sync.dma_start(
                    out=out_r[m, :, 4 * g:4 * g + 4, :]
                    .rearrange("p r w -> p (r w)"),
                    in_=fo)

            # ---- schedule ----
            # V chains for half-block hb run during hb's own rows (slots
            # 0-5 cover rows 2-7; rows 0-1 were issued at slots 6-7 of
            # hb-1), so the last half-block's rows still have PE filler.
            load_xt(0)
            load_xt(1)
            alloc_qk(0)
            alloc_vmix(0)
            for c in qk_chains(0, q_on_dve=True):
                c()
            vchain(0, 0)
            vchain(0, 1)

            pending = []
            odue = []
            for gr in range(RPC):
                hb, r = divmod(gr, HB)
                if r == 0:
                    if hb + 1 < NHB:
                        alloc_qk(hb + 1)
                        pending = qk_chains(hb + 1)
                    else:
                        pending = []
                    if hb + 2 < NHB:
                        load_xt(hb + 2)
                scores(gr)
                for c in pending[2 * r:2 * r + 2]:
                    c()
                if r == 1 and hb + 1 < NHB:
                    alloc_vmix(hb + 1)
                if r <= 5:
                    vchain(hb, r + 2)
                elif hb + 1 < NHB:
                    vchain(hb + 1, r - 6)
                if gr >= 1:
                    av(gr - 1)
                nun = 2 if (pending or hb == 0) else 3
                for _ in range(nun):
                    if odue:
                        g, m = odue.pop(0)
                        oproj_unit(g, m)
                if gr >= 2:
                    transpose(gr - 2)
                    if (gr - 2) % 4 == 3:
                        g = (gr - 2) // 4
                        odue += [(g, m) for m in range(NCH)]
            av(RPC - 1)
            transpose(RPC - 2)
            for g, m in odue[:2]:
                oproj_unit(g, m)
            odue = odue[2:]
            transpose(RPC - 1)
            for g, m in odue:
                oproj_unit(g, m)
            for m in range(NCH):
                oproj_unit(RPC // 4 - 1, m)

    nc.compile()
    return nc


def _get_nc():
    global _CACHED_NC
    if _CACHED_NC is None:
        _CACHED_NC = _build_nc()
    return _CACHED_NC


def kernel(x, q_w, q_b, k_w, k_b, v_w, v_b, o_w, o_b,
           pos_q, pos_k, pos_v, g_q, g_k, g_v1, g_v2):
    global LAST_RESULTS
    x = np.asarray(x, dtype=np.float32)
    q_w = np.asarray(q_w, dtype=np.float32)
    k_w = np.asarray(k_w, dtype=np.float32)
    v_w = np.asarray(v_w, dtype=np.float32)
    o_w = np.asarray(o_w, dtype=np.float32)
    q_b = np.asarray(q_b, dtype=np.float32)
    k_b = np.asarray(k_b, dtype=np.float32)
    v_b = np.asarray(v_b, dtype=np.float32)
    o_b = np.asarray(o_b, dtype=np.float32)
    pq = np.asarray(pos_q, dtype=np.float32)[0, :, :W, :]
    pk = np.asarray(pos_k, dtype=np.float32)[0, :, :W, :]
    pv = np.asarray(pos_v, dtype=np.float32)[0, :, :W, :]
    gq = float(np.asarray(g_q).reshape(-1)[0])
    gk = float(np.asarray(g_k).reshape(-1)[0])
    gv1 = float(np.asarray(g_v1).reshape(-1)[0])
    gv2 = float(np.asarray(g_v2).reshape(-1)[0])

    scale = HD ** (-0.5)

    E4 = ml_dtypes.float8_e4m3
    xt_all = x.transpose(0, 2, 1, 3).reshape(ROWS, C, W).astype(BF16)
    xt8_all = xt_all.astype(E4)
    vw8 = np.ascontiguousarray(v_w.T * (gv1 * 32.0)).astype(E4)
    q_wt = np.ascontiguousarray(q_w.T * scale).astype(BF16)
    k_wt = np.ascontiguousarray(k_w.T).astype(BF16)
    v_wt = np.ascontiguousarray(v_w.T * gv1).astype(BF16)
    o_wt = np.ascontiguousarray(o_w.T).astype(BF16)
    qb_s = (q_b * scale).astype(np.float32)
    kb_s = k_b.astype(np.float32)

    pq_ch = (gq * pq).transpose(0, 2, 1).reshape(NCH, P, W) \
        + k_b.reshape(NCH, P)[:, :, None]
    pqr = np.ascontiguousarray(
        np.tile(pq_ch[:, :, None, :], (1, 1, 4, 1)).reshape(NCH, P, 512)
    ).astype(BF16)

    pkts = np.ascontiguousarray(
        (gk * scale * pk).transpose(0, 2, 1).reshape(NCH, P, W)).astype(BF16)
    pvs = np.ascontiguousarray(
        gv2 * pv.transpose(1, 0, 2).reshape(W, C)
        + gv1 * v_b[None, :]).astype(BF16)
    ident = np.eye(P, dtype=np.float32).astype(BF16)

    shared = {
        "q_wt": q_wt, "k_wt": k_wt, "v_wt": v_wt, "o_wt": o_wt,
        "qb": qb_s, "kb": kb_s,
        "pqr": pqr, "pkts": pkts, "pvs": pvs, "ident": ident,
        "vw8": vw8,
    }
    in_maps = []
    for c in range(NCORES):
        m = dict(shared)
        xs = xt_all[c * RPC:(c + 1) * RPC]
        m["xt"] = np.ascontiguousarray(
            xs.reshape(RPC, NCH, P, W).transpose(1, 2, 0, 3))
        xs8 = xt8_all[c * RPC:(c + 1) * RPC]
        m["xt8"] = np.ascontiguousarray(
            xs8.reshape(RPC, NCH, P, W).transpose(1, 2, 0, 3))
        in_maps.append(m)

    nc = _get_nc()
    res = run_bass_kernel_spmd(nc, in_maps, core_ids=list(range(NCORES)))
    LAST_RESULTS = res

    out_all = np.concatenate(
        [res.results[c]["out_t"].transpose(2, 0, 1, 3).reshape(RPC, C, W)
         for c in range(NCORES)], axis=0)
    y = out_all.reshape(B, H, C, W).transpose(0, 2, 1, 3)
    y = y + o_b[None, :, None, None]
    return np.ascontiguousarray(y.astype(np.float32))
